# revision 26
# baseline (speedup 1.0000x reference)
"""Trainium2 Bass kernel for AssignClsLabel (clipped-IoU >= 0.7 proposal labeling).

Problem: bboxess [8, 65536, 4] f32, gt_bboxess [8, 64, 4] f32,
gt_counts/counts [8,1] int. Output labels [8, 65536, 1] int (0/1).

Sharding: data-parallel, one batch per NeuronCore (8 cores).

Device math (validated bit-exact vs reference on the fixed dataset):
  per (proposal n, gt a):
    r2(y) = relu(d21 - relu(y - gy1))      [= gy2 - clip(y, gy1, gy2)]
    dy = r2(y1) - r2(y2)  (= clip(y2)-clip(y1));  dx likewise with gx
    inter = dy*dx
    u = (area + ga) - inter
    m = (0.7*u - inter) * u       # sign test: iou >= 0.7  <=>  m <= 0
  label = (min_a m <= 0) & (n < count)
Invalid gts (a >= gt_count) are replaced host-side by a far box (2,2,3,3)
which can never fire.

Engine split: ScalarE (ACT) computes ALL clips in relu form (2 ops per
coordinate pair per gt, FD=1024 over the raw interleaved (y1,y2)/(x1,x2)
pairs); VectorE (DVE) does the whole arithmetic core (dy, dx, inter, u,
dp, m, acc-min). ACT and DVE overlap perfectly on TRN2; GPSIMD is avoided
for bulk elementwise (it serializes with DVE on the SBUF ports).
"""
import sys

import numpy as np

if "/opt/trn_rl_repo" not in sys.path:
    sys.path.insert(0, "/opt/trn_rl_repo")

import concourse.mybir as mybir
import concourse.tile as tile
from concourse import bacc
from concourse.bass_utils import run_bass_kernel_spmd

AOP = mybir.AluOpType
ACT = mybir.ActivationFunctionType
F32 = mybir.dt.float32
I32 = mybir.dt.int32

P = 128          # SBUF partitions; proposals n = p*C + c
A = 64           # gt boxes per batch
G = 4            # gts per inner group
BIG = 3.0e38

# scal column layout (all values broadcast to 128 partitions host-side)
COL_GA = 0       # ga = (gy2-gy1)*(gx2-gx1)
COL_NGY1 = 64    # -gy1  (ACT bias for relu(y - gy1))
COL_D21Y = 128   # gy2 - gy1
COL_NGX1 = 192   # -gx1
COL_E21X = 256   # gx2 - gx1
COL_CNT = 320
SCAL_W = 324


def build_graph(C: int):
    """One-core graph; SPMD across 8 cores. C = proposals per partition."""
    NG = A // G
    FD = G * C
    # Bacc (not plain Bass): its finalize() runs generate_event_semaphores,
    # which splits multi-proc sync waits into EventSemaphore instructions —
    # walrus accepts at most ONE wait per ordinary instruction.
    nc = bacc.Bacc()

    bbox_d = nc.declare_dram_parameter("bbox", [P, 4 * C], F32, isOutput=False)
    scal_d = nc.declare_dram_parameter("scal", [P, SCAL_W], F32, isOutput=False)
    iota_d = nc.declare_dram_parameter("iota", [P, C], F32, isOutput=False)
    out_d = nc.declare_dram_parameter("out", [P, 2 * C], I32, isOutput=True)

    with tile.TileContext(nc) as tc:
        with (
            tc.tile_pool(name="persist", bufs=1) as pp,
            tc.tile_pool(name="grp", bufs=1) as gp,
        ):
            braw = pp.tile([P, 4 * C], F32, tag="braw")
            scal = pp.tile([P, SCAL_W], F32, tag="scal")
            iot = pp.tile([P, C], F32, tag="iot")
            nc.sync.dma_start(braw[:], bbox_d[:])
            nc.sync.dma_start(scal[:], scal_d[:])
            nc.sync.dma_start(iot[:], iota_d[:])

            # raw free idx = 4c + 2j + i : i=0 -> y coords, i=1 -> x coords;
            # j=0 -> (y1,x1), j=1 -> (y2,x2)
            bv = braw[:].rearrange("p (c j i) -> p c j i", j=2, i=2)
            ypair = bv[:, :, :, 0]          # [P, C, 2] = (y1, y2), steps (4,2)
            xpair = bv[:, :, :, 1]          # [P, C, 2] = (x1, x2)
            y1v, y2v = bv[:, :, 0, 0], bv[:, :, 1, 0]
            x1v, x2v = bv[:, :, 0, 1], bv[:, :, 1, 1]

            area = pp.tile([P, C], F32, tag="area")
            tdy = pp.tile([P, C], F32, tag="tdy")
            nc.vector.tensor_tensor(tdy[:], y2v, y1v, AOP.subtract)
            nc.vector.tensor_tensor(area[:], x2v, x1v, AOP.subtract)
            nc.vector.tensor_tensor(area[:], tdy[:], area[:], AOP.mult)

            acc = [pp.tile([P, FD], F32, tag=f"acc{i}", name=f"acc{i}")
                   for i in range(2)]
            nc.vector.memset(acc[0][:], BIG)

            # ACT's per-instruction sync-wait budget is 1: give it DVE-produced
            # scalars so its data deps collapse onto the DVE proc.
            scal2 = pp.tile([P, SCAL_W], F32, tag="scal2")
            nc.vector.tensor_copy(scal2[:], scal[:])

            def gcol(base, a):
                return scal2[:, base + a : base + a + 1]

            for g in range(NG):
                r2ps = []
                s2ps = []
                for j in range(G):
                    a = g * G + j
                    # ACT: paired relu clips, FD=1024 per op
                    r1p = gp.tile([P, 2 * C], F32, tag="r1p", bufs=2)
                    r2p = gp.tile([P, 2 * C], F32, tag="r2p", bufs=2,
                                  name=f"r2p_{g}_{j}")
                    s1p = gp.tile([P, 2 * C], F32, tag="s1p", bufs=2)
                    s2p = gp.tile([P, 2 * C], F32, tag="s2p", bufs=2,
                                  name=f"s2p_{g}_{j}")
                    r1pv = r1p[:].rearrange("p (c j) -> p c j", j=2)
                    s1pv = s1p[:].rearrange("p (c j) -> p c j", j=2)
                    nc.scalar.activation(
                        r1pv, ypair, ACT.Relu, bias=gcol(COL_NGY1, a))
                    nc.scalar.activation(
                        r2p[:], r1p[:], ACT.Relu,
                        bias=gcol(COL_D21Y, a), scale=-1.0)
                    nc.scalar.activation(
                        s1pv, xpair, ACT.Relu, bias=gcol(COL_NGX1, a))
                    nc.scalar.activation(
                        s2p[:], s1p[:], ACT.Relu,
                        bias=gcol(COL_E21X, a), scale=-1.0)
                    r2ps.append(r2p)
                    s2ps.append(s2p)

                dy = gp.tile([P, FD], F32, tag="dy")
                dx = gp.tile([P, FD], F32, tag="dx")
                for j in range(G):
                    s = slice(j * C, (j + 1) * C)
                    r2v = r2ps[j][:].rearrange("p (c j) -> p j c", j=2)
                    s2v = s2ps[j][:].rearrange("p (c j) -> p j c", j=2)
                    # dy = r2(y1) - r2(y2), dx = s2(x1) - s2(x2)
                    nc.vector.tensor_tensor(
                        dy[:, s], r2v[:, 0, :], r2v[:, 1, :], AOP.subtract)
                    nc.vector.tensor_tensor(
                        dx[:, s], s2v[:, 0, :], s2v[:, 1, :], AOP.subtract)

                inter = gp.tile([P, FD], F32, tag="inter")
                nc.vector.tensor_tensor(inter[:], dy[:], dx[:], AOP.mult)

                u = gp.tile([P, FD], F32, tag="u")
                dp = gp.tile([P, FD], F32, tag="dp")
                for j in range(G):
                    a = g * G + j
                    s = slice(j * C, (j + 1) * C)
                    nc.vector.scalar_tensor_tensor(
                        u[:, s], area[:], gcol(COL_GA, a), inter[:, s],
                        AOP.add, AOP.subtract)
                nc.vector.scalar_tensor_tensor(
                    dp[:], u[:], 0.7, inter[:], AOP.mult, AOP.subtract)

                m = gp.tile([P, FD], F32, tag="m")
                nc.vector.tensor_tensor(m[:], dp[:], u[:], AOP.mult)
                nc.vector.tensor_tensor(
                    acc[(g + 1) % 2][:], acc[g % 2][:], m[:], AOP.min)

            accfin = acc[NG % 2]
            accv = accfin[:].rearrange("p (a c) -> p c a", a=G)
            macc = pp.tile([P, C], F32, tag="macc")
            nc.vector.tensor_reduce(macc[:], accv, mybir.AxisListType.X, AOP.min)

            vb = pp.tile([P, C], F32, tag="vb")
            nc.vector.tensor_scalar(
                vb[:], iot[:], scal[:, COL_CNT:COL_CNT + 1], None, AOP.is_lt)
            lblf = pp.tile([P, C], F32, tag="lblf")
            nc.vector.scalar_tensor_tensor(
                lblf[:], macc[:], 0.0, vb[:], AOP.is_le, AOP.mult)

            outsb = pp.tile([P, 2 * C], I32, tag="outsb")
            nc.vector.memset(outsb[:], 0)
            oview = outsb[:].rearrange("p (c k) -> p k c", k=2)
            nc.vector.tensor_copy(oview[:, 0, :], lblf[:])
            nc.sync.dma_start(out_d[:], outsb[:])

    nc.finalize()
    return nc


def host_prep(bboxess, gt_bboxess, gt_counts, counts, C):
    """Per-core input shards. Core b gets batch b."""
    B, N, _ = bboxess.shape
    assert N == P * C
    iota = np.arange(N, dtype=np.float32).reshape(P, C)
    in_maps = []
    for b in range(B):
        bb = np.ascontiguousarray(
            bboxess[b].astype(np.float32).reshape(P, 4 * C))
        g = gt_bboxess[b].astype(np.float32).copy()
        nv = int(gt_counts[b, 0])
        g[nv:, 0] = 2.0
        g[nv:, 1] = 2.0
        g[nv:, 2] = 3.0
        g[nv:, 3] = 3.0
        gy1, gx1, gy2, gx2 = g[:, 0], g[:, 1], g[:, 2], g[:, 3]
        row = np.zeros(SCAL_W, dtype=np.float32)
        row[COL_GA:COL_GA + 64] = (gy2 - gy1) * (gx2 - gx1)
        row[COL_NGY1:COL_NGY1 + 64] = -gy1
        row[COL_D21Y:COL_D21Y + 64] = gy2 - gy1
        row[COL_NGX1:COL_NGX1 + 64] = -gx1
        row[COL_E21X:COL_E21X + 64] = gx2 - gx1
        row[COL_CNT] = float(int(counts[b, 0]))
        scal = np.ascontiguousarray(np.broadcast_to(row, (P, SCAL_W)))
        in_maps.append({"bbox": bb, "scal": scal, "iota": iota})
    return in_maps


def _axon_reset():
    import ctypes
    try:
        lib = ctypes.CDLL("/opt/axon/libaxon_pjrt.so")
        lib.axon_reset.restype = ctypes.c_int64
        lib.axon_reset()
    except Exception:
        pass


def kernel(bboxess, gt_bboxess, gt_counts, counts):
    B, N, _ = bboxess.shape
    C = N // P
    nc = build_graph(C)
    in_maps = host_prep(bboxess, gt_bboxess, gt_counts, counts, C)
    try:
        res = run_bass_kernel_spmd(nc, in_maps, core_ids=list(range(B)))
    except Exception:
        _axon_reset()
        res = run_bass_kernel_spmd(nc, in_maps, core_ids=list(range(B)))
    out_dtype = np.int64 if counts.dtype == np.int64 else np.int32
    labels = np.empty((B, N, 1), dtype=out_dtype)
    for b in range(B):
        o = res.results[b]["out"]                    # [P, 2C] int32
        pairs = o.reshape(P, C, 2)
        if out_dtype == np.int64:
            labels[b] = pairs.view(np.int64).reshape(N, 1)
        else:
            labels[b] = np.ascontiguousarray(pairs[:, :, 0]).reshape(N, 1)
    return labels
